# revision 2
# baseline (speedup 1.0000x reference)
"""Distributed 2-layer GCN (DGL GraphConv norm='both') + mean-pool head on 8 TRN2 NeuronCores.

Strategy
--------
GraphConv is linear, so fold both degree normalizations into per-edge weights
w_e = rsqrt(deg_out[src_e]) * rsqrt(deg_in[dst_e]) and reorder each layer as
transform-then-aggregate:

    t = x @ W                       (dense matmul, node-sharded across cores)
    agg[d] = sum_e w_e * t[src_e]   (sparse aggregation, dst-sharded)
    feat = relu(agg + b)            (leaky_relu after relu is a no-op)

Sharding: nodes are split into 8 contiguous shards; core k computes t-rows and
aggregations for its shard. After each transform the t-shards are AllGathered
so every core can gather arbitrary source rows.

Aggregation on device: edges are sorted by dst and grouped into 128-dst-node
blocks (host-side, index-only preprocessing). For each block, source rows are
fetched with one indirect DMA (row gather), and the scatter-add becomes a
dense matmul: for each chunk of 128 edges, build S[e, slot] =
(iota[slot] == dst_slot_e) * w_e with a single fused tensor_scalar op, then
PSUM-accumulate G_chunk.T @ S -> agg^T [feat, dst_slot]. The transposed layout
makes the bias a per-partition ACT bias and feeds the next layer's matmul
(lhsT = relu(agg^T + b)) with no transposes anywhere.

Readout: ACT accum_out gives the free-axis (dst) sum of relu() for free;
per-core partials are AllReduced, and the tiny MLP head runs in column form
(out = W.T @ col) on every core redundantly.
"""

import sys

sys.path.insert(0, "/opt/trn_rl_repo")

import numpy as np

import concourse.bacc as bacc
import concourse.bass as bass
import concourse.mybir as mybir
import concourse.tile as tile
from concourse.bass import IndirectOffsetOnAxis
from concourse.bass_utils import run_bass_kernel_spmd

NCORES = 8
P = 128
CPAD = 16
LEAKY = 0.01
F32 = mybir.dt.float32
BF16 = mybir.dt.bfloat16
I32 = mybir.dt.int32


def _dims(N, E):
    assert N % NCORES == 0
    shard = N // NCORES
    nblk = -(-shard // P)
    shard_pad = nblk * P
    bt = max(d for d in range(1, 33) if nblk % d == 0)
    valid_last = shard - (nblk - 1) * P
    return shard, nblk, shard_pad, bt, valid_last


def preprocess(x, src, dst):
    """Index-only host preprocessing: degree rsqrt folding, edge partitioning
    by (dst-shard, dst-block), uniform chunk padding, per-core input maps."""
    N, D = x.shape
    E = src.shape[0]
    assert D == P
    shard, nblk, shard_pad, bt, valid_last = _dims(N, E)

    src = np.asarray(src).astype(np.int64)
    dst = np.asarray(dst).astype(np.int64)

    deg_out = np.bincount(src, minlength=N).astype(np.float32)
    deg_in = np.bincount(dst, minlength=N).astype(np.float32)
    r_out = (1.0 / np.sqrt(np.maximum(deg_out, 1.0))).astype(np.float32)
    r_in = (1.0 / np.sqrt(np.maximum(deg_in, 1.0))).astype(np.float32)

    order = np.argsort(dst, kind="stable")
    ds = dst[order]
    ss = src[order]

    gid = (ds // shard) * nblk + (ds % shard) // P
    ngrp = NCORES * nblk
    counts = np.bincount(gid, minlength=ngrp)
    K1 = max(1, int(-(-counts.max() // P)))
    CAP = K1 * P

    starts = np.zeros(ngrp, np.int64)
    starts[1:] = np.cumsum(counts)[:-1]
    pos = np.arange(E, dtype=np.int64) - starts[gid]
    flat = gid * CAP + pos

    pid = ((ss // shard) * shard_pad + (ss % shard)).astype(np.int32)
    idx_flat = np.zeros(ngrp * CAP, np.int32)
    slot_flat = np.zeros(ngrp * CAP, np.float32)
    w_flat = np.zeros(ngrp * CAP, np.float32)
    idx_flat[flat] = pid
    slot_flat[flat] = ((ds % shard) % P).astype(np.float32)
    w_flat[flat] = r_out[ss] * r_in[ds]

    idx_a = idx_flat.reshape(NCORES, nblk, P, K1)
    slot_a = slot_flat.reshape(NCORES, nblk, P, K1)
    w_a = w_flat.reshape(NCORES, nblk, P, K1)

    # per-core transposed x shard, zero-padded to shard_pad columns
    xT = np.zeros((NCORES, P, shard_pad), np.float32)
    xv = np.ascontiguousarray(x.astype(np.float32))
    for k in range(NCORES):
        xT[k, :, :shard] = xv[k * shard : (k + 1) * shard].T

    iota = np.tile(np.arange(P, dtype=np.float32), (P, 1))
    return dict(
        N=N, E=E, shard=shard, nblk=nblk, shard_pad=shard_pad, bt=bt,
        valid_last=valid_last, K1=K1, xT=xT, idx=idx_a, slot=slot_a, w=w_a,
        iota=iota,
    )


def build_nc(N, nblk, shard_pad, bt, valid_last, K1):
    """Build the SPMD Bass program (same program for all 8 cores)."""
    CAP = K1 * P
    rg = [list(range(NCORES))]
    nc = bacc.Bacc("TRN2", target_bir_lowering=False, debug=False,
                   num_devices=NCORES)

    xT_p = nc.declare_dram_parameter("xT", [P, shard_pad], F32, False)
    w0_p = nc.declare_dram_parameter("W0", [P, P], F32, False)
    w1_p = nc.declare_dram_parameter("W1", [P, P], F32, False)
    wl1_p = nc.declare_dram_parameter("WL1", [P, P], F32, False)
    wl2_p = nc.declare_dram_parameter("WL2", [P, CPAD], F32, False)
    b0_p = nc.declare_dram_parameter("b0", [P, 1], F32, False)
    b1_p = nc.declare_dram_parameter("b1", [P, 1], F32, False)
    bl1_p = nc.declare_dram_parameter("bL1", [P, 1], F32, False)
    bl2_p = nc.declare_dram_parameter("bL2", [CPAD, 1], F32, False)
    iota_p = nc.declare_dram_parameter("iota", [P, P], F32, False)
    idx_p = nc.declare_dram_parameter("idx", [nblk, P, K1], I32, False)
    slot_p = nc.declare_dram_parameter("slot", [nblk, P, K1], F32, False)
    wgt_p = nc.declare_dram_parameter("wgt", [nblk, P, K1], F32, False)
    y_p = nc.declare_dram_parameter("y", [CPAD, 1], F32, True)

    with tile.TileContext(nc) as tc:
        with (
            tc.tile_pool(name="consts", bufs=1) as consts,
            tc.tile_pool(name="xin", bufs=2) as xin,
            tc.tile_pool(name="stg", bufs=3) as stg,
            tc.tile_pool(name="auxp", bufs=3) as auxp,
            tc.tile_pool(name="gp", bufs=2) as gp,
            tc.tile_pool(name="sp", bufs=4) as sp,
            tc.tile_pool(name="hp", bufs=2) as hp,
            tc.tile_pool(name="misc", bufs=1) as misc,
            tc.tile_pool(name="psA", bufs=2, space="PSUM") as psA,
            tc.tile_pool(name="psB", bufs=2, space="PSUM") as psB,
            tc.tile_pool(name="dram", bufs=1, space="DRAM") as dram,
        ):
            # ---- constants ----
            w0bf = consts.tile([P, P], BF16)
            nc.gpsimd.dma_start(w0bf[:], w0_p[:])  # f32 -> bf16 cast DMA
            w1bf = consts.tile([P, P], BF16)
            nc.gpsimd.dma_start(w1bf[:], w1_p[:])
            wl1sb = consts.tile([P, P], F32)
            nc.sync.dma_start(wl1sb[:], wl1_p[:])
            wl2sb = consts.tile([P, CPAD], F32)
            nc.sync.dma_start(wl2sb[:], wl2_p[:])
            b0c = consts.tile([P, 1], F32)
            nc.sync.dma_start(b0c[:], b0_p[:])
            b1c = consts.tile([P, 1], F32)
            nc.sync.dma_start(b1c[:], b1_p[:])
            bl1c = consts.tile([P, 1], F32)
            nc.sync.dma_start(bl1c[:], bl1_p[:])
            bl2c = consts.tile([CPAD, 1], F32)
            nc.sync.dma_start(bl2c[:], bl2_p[:])
            iota_f = consts.tile([P, P], F32)
            nc.sync.dma_start(iota_f[:], iota_p[:])
            iota_sb = consts.tile([P, P], BF16)
            nc.vector.tensor_copy(iota_sb[:], iota_f[:])

            t0loc = dram.tile([shard_pad, P], BF16)
            t0full = dram.tile([NCORES * shard_pad, P], BF16, addr_space="Shared")
            t1loc = dram.tile([shard_pad, P], BF16)
            t1full = dram.tile([NCORES * shard_pad, P], BF16, addr_space="Shared")
            arin = dram.tile([P, 1], F32)
            arout = dram.tile([P, 1], F32, addr_space="Shared")

            # ---- phase A: t0 shard = x_shard @ W0 (bf16) ----
            for t in range(nblk // bt):
                xsb = xin.tile([P, bt * P], BF16, tag="xsb")
                nc.gpsimd.dma_start(
                    xsb[:], xT_p[:, t * bt * P : (t + 1) * bt * P]
                )
                for i in range(bt):
                    b = t * bt + i
                    pt0 = psA.tile([P, P], F32, space="PSUM", tag="pt0")
                    nc.tensor.matmul(
                        pt0[:], lhsT=xsb[:, i * P : (i + 1) * P], rhs=w0bf[:],
                        start=True, stop=True,
                    )
                    st = stg.tile([P, P], BF16, tag="st")
                    nc.vector.tensor_copy(st[:], pt0[:])
                    nc.sync.dma_start(t0loc[b * P : (b + 1) * P, :], st[:])

            nc.gpsimd.collective_compute(
                "AllGather", mybir.AluOpType.bypass, replica_groups=rg,
                ins=[t0loc.opt()], outs=[t0full.opt()],
            )

            def agg_layer(tfull, bias_col, produce, acc_tile):
                """Aggregate over this core's dst blocks, gathering from tfull.
                produce=True: write relu-layer t-transform into t1loc.
                produce=False: accumulate readout sums into acc_tile."""
                for b in range(nblk):
                    isb = auxp.tile([P, K1], I32, tag="isb")
                    nc.sync.dma_start(isb[:], idx_p[b])
                    ssb = auxp.tile([P, K1], F32, tag="ssb")
                    nc.sync.dma_start(ssb[:], slot_p[b])
                    wsb = auxp.tile([P, K1], F32, tag="wsb")
                    nc.sync.dma_start(wsb[:], wgt_p[b])
                    G = gp.tile([P, CAP], BF16, tag="G")
                    pagg = psB.tile([P, P], F32, space="PSUM", tag="pagg")
                    for j in range(K1):
                        nc.gpsimd.indirect_dma_start(
                            out=G[:, j * P : (j + 1) * P], out_offset=None,
                            in_=tfull,
                            in_offset=IndirectOffsetOnAxis(
                                ap=isb[:, j : j + 1], axis=0
                            ),
                        )
                        S = sp.tile([P, P], BF16, tag="S")
                        nc.vector.tensor_scalar(
                            out=S[:], in0=iota_sb[:],
                            scalar1=ssb[:, j : j + 1], scalar2=wsb[:, j : j + 1],
                            op0=mybir.AluOpType.is_equal, op1=mybir.AluOpType.mult,
                        )
                        nc.tensor.matmul(
                            pagg[:], lhsT=G[:, j * P : (j + 1) * P], rhs=S[:],
                            start=(j == 0), stop=(j == K1 - 1),
                        )
                    if produce:
                        h1 = hp.tile([P, P], BF16, tag="h1")
                        nc.scalar.activation(
                            out=h1[:], in_=pagg[:],
                            func=mybir.ActivationFunctionType.Relu, bias=bias_col,
                        )
                        pt1 = psA.tile([P, P], F32, space="PSUM", tag="pt0")
                        nc.tensor.matmul(
                            pt1[:], lhsT=h1[:], rhs=w1bf[:], start=True, stop=True
                        )
                        st1 = stg.tile([P, P], BF16, tag="st")
                        nc.vector.tensor_copy(st1[:], pt1[:])
                        nc.sync.dma_start(t1loc[b * P : (b + 1) * P, :], st1[:])
                    else:
                        nv = valid_last if b == nblk - 1 else P
                        dead = hp.tile([P, P], BF16, tag="h1")
                        nc.scalar.activation(
                            out=dead[:, :nv], in_=pagg[:, :nv],
                            func=mybir.ActivationFunctionType.Relu, bias=bias_col,
                            accum_out=acc_tile[:, b : b + 1],
                        )

            agg_layer(t0full[:, :], b0c[:, 0:1], True, None)

            nc.gpsimd.collective_compute(
                "AllGather", mybir.AluOpType.bypass, replica_groups=rg,
                ins=[t1loc.opt()], outs=[t1full.opt()],
            )

            acc = misc.tile([P, nblk], F32)
            agg_layer(t1full[:, :], b1c[:, 0:1], False, acc)

            # ---- readout: mean-pool + tiny MLP head (column form) ----
            partial = misc.tile([P, 1], F32)
            nc.vector.tensor_reduce(
                out=partial[:], in_=acc[:], axis=mybir.AxisListType.X,
                op=mybir.AluOpType.add,
            )
            nc.sync.dma_start(arin[:], partial[:])
            nc.gpsimd.collective_compute(
                "AllReduce", mybir.AluOpType.add, replica_groups=rg,
                ins=[arin.opt()], outs=[arout.opt()],
            )
            mr = misc.tile([P, 1], F32)
            nc.sync.dma_start(mr[:], arout[:])
            mc = misc.tile([P, 1], F32)
            nc.vector.tensor_scalar_mul(mc[:], mr[:], 1.0 / float(N))
            ph = psB.tile([P, 1], F32, space="PSUM", tag="ph")
            nc.tensor.matmul(ph[:], lhsT=wl1sb[:], rhs=mc[:], start=True, stop=True)
            z = misc.tile([P, 1], F32)
            nc.vector.tensor_scalar(
                out=z[:], in0=ph[:], scalar1=bl1c[:, 0:1], scalar2=None,
                op0=mybir.AluOpType.add,
            )
            za = misc.tile([P, 1], F32)
            nc.vector.tensor_scalar_mul(za[:], z[:], LEAKY)
            hg = misc.tile([P, 1], F32)
            nc.vector.tensor_tensor(
                out=hg[:], in0=z[:], in1=za[:], op=mybir.AluOpType.max
            )
            po = psB.tile([P, 1], F32, space="PSUM", tag="ph")
            nc.tensor.matmul(
                po[:CPAD, :], lhsT=wl2sb[:], rhs=hg[:], start=True, stop=True
            )
            yv = misc.tile([CPAD, 1], F32)
            nc.vector.tensor_scalar(
                out=yv[:], in0=po[:CPAD, :], scalar1=bl2c[:, 0:1], scalar2=None,
                op0=mybir.AluOpType.add,
            )
            nc.sync.dma_start(y_p[:], yv[:])

    nc.compile()
    return nc


def make_in_maps(hd, W0, b0, W1, b1, WL1, bL1, WL2, bL2):
    C = WL2.shape[1]
    wl2p = np.zeros((P, CPAD), np.float32)
    wl2p[:, :C] = np.asarray(WL2, np.float32)
    bl2c = np.zeros((CPAD, 1), np.float32)
    bl2c[:C, 0] = np.asarray(bL2, np.float32)
    shared = dict(
        W0=np.asarray(W0, np.float32), W1=np.asarray(W1, np.float32),
        WL1=np.asarray(WL1, np.float32), WL2=wl2p,
        b0=np.asarray(b0, np.float32).reshape(P, 1),
        b1=np.asarray(b1, np.float32).reshape(P, 1),
        bL1=np.asarray(bL1, np.float32).reshape(P, 1), bL2=bl2c,
        iota=hd["iota"],
    )
    return [
        dict(shared, xT=hd["xT"][k], idx=hd["idx"][k], slot=hd["slot"][k],
             wgt=hd["w"][k])
        for k in range(NCORES)
    ]


def kernel(x, src, dst, W0, b0, W1, b1, WL1, bL1, WL2, bL2):
    x = np.asarray(x)
    hd = preprocess(x, np.asarray(src), np.asarray(dst))
    nc = build_nc(hd["N"], hd["nblk"], hd["shard_pad"], hd["bt"],
                  hd["valid_last"], hd["K1"])
    in_maps = make_in_maps(hd, W0, b0, W1, b1, WL1, bL1, WL2, bL2)
    res = run_bass_kernel_spmd(nc, in_maps, list(range(NCORES)))
    C = np.asarray(WL2).shape[1]
    return res.results[0]["y"][:C, 0].reshape(1, C).astype(np.float32)
